# revision 16
# baseline (speedup 1.0000x reference)
"""Trainium2 Bass kernel for the grouped contrastive loss.

Math: for anchors i and positives j restricted to the same
sensitive-attribute group g (size P),
    row(i,j) = S_ij - D * log E_ij
with S_ij = <p_i, p_j>/t and E_ij = sum_d exp(p_i[d] p_j[d] / t)
(the log-softmax max-shift cancels analytically), and
    loss = sum_i -1/(N P_i^2) * sum_{j in g(i)} row(i,j).

row(i,j) is symmetric, so the group's P x P matrix is covered by
chunking each group into <=128-column chunks: the diagonal chunk-square
is computed in full at weight 1 and cross chunk pairs only once (rows of
earlier chunks x cols of later chunk) at weight 2 -- B(B+1)/2 slots per
group instead of B^2. Slot = up to 128 anchor rows x one col chunk
(W=128), rows packed 128-at-a-time from all chunks <= the col chunk.

Per slot, on device (anchors on partitions as 32 packs of 4 anchors x
32 dims):
  - S via one bf16 matmul (lhsT = anchor points [32,128], rhs = window
    points [32,128]).
  - prod via ONE DVE tensor_tensor per slot: scalars stored duplicated
    in pairs (scal2[p,2k]=scal2[p,2k+1]) so all three operands' APs end
    in a packed [1,2] bf16 dim -> DVE 2x mode; stride-0 outer dims do
    the pack/window broadcast. Then one batched ACT Exp ([128, 4096]
    bf16), and per-pack bf16 matmuls against shifted block-diagonal
    ones accumulating each anchor's 32 exp rows into its PSUM row
    (4 chains x 8 packs into one [128,128] PSUM tile via explicit
    tile_position).
  - Ln on ACT, then DVE row-reductions of log E and S; weighted
    accumulate into a [128] per-core partial.
A manually pre-placed InstLoadActFuncSet of the combined exp+ln table
avoids the per-switch ACT table reloads. Dummy rows/cols are weighted
out (w=0) or corrected by the exact constant D*ln(D)*n_dummy per slot.
The 8 cores run one SPMD program; the host sums the [128] partials.
"""

import math
import os
import sys

sys.path.insert(0, "/opt/trn_rl_repo")

import numpy as np
import ml_dtypes

import concourse.bacc as bacc
import concourse.bass as bass
import concourse.tile as tile
from concourse import mybir
from concourse.bass_utils import run_bass_kernel_spmd

N_CORES = 8
D = 32
W = 128  # window (col chunk) width
PACKS = 32  # packs of 4 anchors per 128-anchor slot

last_run_info = {}


def _install_ntff_hook():
    # bass_utils' trace path under axon imports antenv.axon_hooks, which is
    # absent in this image; provide the ctypes-based hook it expects.
    import contextlib
    import ctypes
    import types

    if "antenv.axon_hooks" in sys.modules:
        return

    def _make_hook():
        try:
            lib = ctypes.CDLL("/opt/axon/libaxon_pjrt.so")
        except OSError:
            return None
        if not hasattr(lib, "axon_start_nrt_profile"):
            return None
        lib.axon_start_nrt_profile.argtypes = [
            ctypes.POINTER(ctypes.c_int64),
            ctypes.c_size_t,
        ]
        lib.axon_start_nrt_profile.restype = ctypes.c_int64
        lib.axon_stop_nrt_profile.argtypes = [ctypes.c_char_p]
        lib.axon_stop_nrt_profile.restype = ctypes.c_int64

        @contextlib.contextmanager
        def _hook_cm(output_dir, device_ids):
            import jax

            jax.devices()
            if device_ids:
                ids = (ctypes.c_int64 * len(device_ids))(*device_ids)
                rc = lib.axon_start_nrt_profile(ids, len(device_ids))
            else:
                rc = lib.axon_start_nrt_profile(None, 0)
            if rc != 0:
                raise RuntimeError(f"axon_start_nrt_profile rc={rc}")
            try:
                yield
            finally:
                n = lib.axon_stop_nrt_profile(str(output_dir).encode())
                if n < 0:
                    raise RuntimeError(f"axon_stop_nrt_profile rc={n}")

        return _hook_cm

    hook = _make_hook()
    mod = types.ModuleType("antenv.axon_hooks")
    mod.get_axon_ntff_profile_hook = lambda: hook
    mod.set_axon_ntff_profile_hook = lambda h: None
    sys.modules["antenv.axon_hooks"] = mod


def _plan(sa_sorted):
    """Slot plan from the sorted attribute vector.

    Each slot is (rows, weights, c0, L):
      rows: array of <=128 sorted-anchor positions (the slot's anchors)
      weights: per-row pair multiplicity (1 diag chunk, 2 earlier chunk)
      [c0, c0+L): the slot's col window (sorted positions, one chunk)
    or None for a dummy slot. Returns (ntiles, per_core).
    """
    n = len(sa_sorted)
    bounds = [0]
    for i in range(1, n):
        if sa_sorted[i] != sa_sorted[i - 1]:
            bounds.append(i)
    bounds.append(n)

    slots = []
    for gi in range(len(bounds) - 1):
        g0, g1 = bounds[gi], bounds[gi + 1]
        P = g1 - g0
        B = (P + W - 1) // W
        for w in range(B):
            c0 = g0 + W * w
            L = min(W, g1 - c0)
            r_hi = min(g0 + W * (w + 1), g1)  # rows of chunks 0..w
            rows_all = np.arange(g0, r_hi)
            wts_all = np.where(rows_all < c0, 2.0, 1.0)
            for r0 in range(0, len(rows_all), 128):
                slots.append(
                    (rows_all[r0 : r0 + 128], wts_all[r0 : r0 + 128], c0, L)
                )

    ntiles = (len(slots) + N_CORES - 1) // N_CORES
    per_core = [[] for _ in range(N_CORES)]
    for i, s in enumerate(slots):
        per_core[i % N_CORES].append(s)
    for c in range(N_CORES):
        while len(per_core[c]) < ntiles:
            per_core[c].append(None)
    return ntiles, per_core


def _exp_ln_table_id(nc):
    try:
        from concourse.hw_specs import get_activation_tables

        tabs = get_activation_tables(nc.m.arch)
        Exp = mybir.ActivationFunctionType.Exp
        Ln = mybir.ActivationFunctionType.Ln
        for idx, funcs in enumerate(tabs.values()):
            if Exp in funcs and Ln in funcs:
                return idx
    except Exception:
        pass
    return 6  # natural_log_exp_and_others in this neuronxcc's act_info.json


def _build_program(ntiles):
    # Bacc (not raw Bass): its compile() runs generate_event_semaphores,
    # which splits multi-semaphore waits to satisfy the TRN2 one-wait-per-
    # instruction constraint this walrus build enforces.
    nc = bacc.Bacc(
        "TRN2", target_bir_lowering=False, debug=False, num_devices=N_CORES
    )
    f32 = mybir.dt.float32
    bf16 = mybir.dt.bfloat16

    rep4_d = nc.dram_tensor("rep4", [128, ntiles * W], bf16, kind="ExternalInput").ap()
    winj_d = nc.dram_tensor("winj", [32, ntiles * W], bf16, kind="ExternalInput").ap()
    lhsa_d = nc.dram_tensor("lhsa", [32, ntiles * 128], bf16, kind="ExternalInput").ap()
    scal_d = nc.dram_tensor(
        "scal2", [128, ntiles * 2 * PACKS], bf16, kind="ExternalInput"
    ).ap()
    # wk packs wcol (cols 0:ntiles) and kcol (cols ntiles:2*ntiles)
    wk_d = nc.dram_tensor("wk", [128, 2 * ntiles], f32, kind="ExternalInput").ap()
    ones_d = nc.dram_tensor("onesbd", [128, 8 * 32], bf16, kind="ExternalInput").ap()
    out_d = nc.dram_tensor("out", [128, 1], f32, kind="ExternalOutput").ap()

    Exp = mybir.ActivationFunctionType.Exp
    Ln = mybir.ActivationFunctionType.Ln

    with tile.TileContext(nc) as tc:
        with (
            tc.tile_pool(name="const", bufs=1) as cpool,
            tc.tile_pool(name="work", bufs=3) as wpool,
            tc.tile_pool(name="red", bufs=3) as rpool,
            tc.tile_pool(name="psE", bufs=3, space="PSUM") as psE,
            tc.tile_pool(name="psS", bufs=3, space="PSUM") as psS,
            tc.tile_pool(name="psL", bufs=2, space="PSUM") as psL,
        ):
            # preload the combined exp+ln table so Exp/Ln interleaving
            # never reloads activation tables (saves ~1.3us per switch)
            nc.scalar.add_instruction(
                mybir.InstLoadActFuncSet(
                    name=nc.get_next_instruction_name(),
                    ins=[],
                    outs=[],
                    act_func_set_id=_exp_ln_table_id(nc),
                )
            )

            # slot-0 slices land first (parallel issue on gpsimd/sync) so
            # compute starts before the bulk transfers
            rep4 = cpool.tile([128, ntiles * W], bf16, tag="rep4")
            nc.gpsimd.dma_start(rep4[:, 0:W], rep4_d[:, 0:W])
            scal = cpool.tile([128, ntiles * 2 * PACKS], bf16, tag="scal2")
            nc.sync.dma_start(scal[:, 0 : 2 * PACKS], scal_d[:, 0 : 2 * PACKS])
            nc.gpsimd.dma_start(rep4[:, W:], rep4_d[:, W:])
            nc.sync.dma_start(scal[:, 2 * PACKS :], scal_d[:, 2 * PACKS :])
            winj = cpool.tile([32, ntiles * W], bf16, tag="winj")
            nc.scalar.dma_start(winj[:], winj_d[:])
            lhsa = cpool.tile([32, ntiles * 128], bf16, tag="lhsa")
            nc.scalar.dma_start(lhsa[:], lhsa_d[:])
            onesbd = cpool.tile([128, 8 * 32], bf16, tag="onesbd")
            nc.gpsimd.dma_start(onesbd[:], ones_d[:])
            wk = cpool.tile([128, 2 * ntiles], f32, tag="wk")
            nc.sync.dma_start(wk[:], wk_d[:])
            wcol = wk[:, 0:ntiles]
            kcol = wk[:, ntiles : 2 * ntiles]

            acc = cpool.tile([128, 1], f32, tag="acc")
            nc.vector.memset(acc[:], 0.0)

            for s in range(ntiles):
                S_ps = psS.tile([128, W], f32, tag="S")
                nc.tensor.matmul(
                    S_ps[:],
                    lhsT=lhsa[:, s * 128 : (s + 1) * 128],
                    rhs=winj[:, s * W : (s + 1) * W],
                    start=True,
                    stop=True,
                )
                prod = wpool.tile([128, PACKS * W], bf16, tag="prod")
                in0 = (
                    rep4[:, s * W : (s + 1) * W]
                    .rearrange("p (j2 two) -> p j2 two", two=2)
                    .unsqueeze(1)
                    .broadcast_to([128, PACKS, W // 2, 2])
                )
                in1 = (
                    scal[:, s * 2 * PACKS : (s + 1) * 2 * PACKS]
                    .rearrange("p (k two) -> p k two", two=2)
                    .unsqueeze(2)
                    .broadcast_to([128, PACKS, W // 2, 2])
                )
                outp = prod[:].rearrange(
                    "p (k j2 two) -> p k j2 two", k=PACKS, two=2
                )
                nc.vector.tensor_tensor(outp, in0, in1, op=mybir.AluOpType.mult)
                expt = wpool.tile([128, PACKS * W], bf16, tag="expt")
                nc.scalar.activation(expt[:], prod[:], Exp)
                E_ps = psE.tile([128, W], f32, tag="E")
                for h in range(4):
                    for i in range(8):
                        k = 8 * h + i
                        nc.tensor.matmul(
                            E_ps[32 * h : 32 * h + 32, :],
                            lhsT=onesbd[:, 32 * i : 32 * (i + 1)],
                            rhs=expt[:, k * W : (k + 1) * W],
                            start=(i == 0),
                            stop=(i == 7),
                            tile_position=(0, 32 * h),
                        )
                logE = psL.tile([128, W], f32, tag="logE")
                nc.scalar.activation(logE[:], E_ps[:], Ln)
                sL = rpool.tile([128, 1], f32, tag="sL")
                nc.vector.tensor_reduce(
                    sL[:], logE[:], axis=mybir.AxisListType.X, op=mybir.AluOpType.add
                )
                sS = rpool.tile([128, 1], f32, tag="sS")
                nc.vector.tensor_reduce(
                    sS[:], S_ps[:], axis=mybir.AxisListType.X, op=mybir.AluOpType.add
                )
                v1 = rpool.tile([128, 1], f32, tag="v1")
                nc.vector.tensor_scalar(
                    v1[:],
                    sL[:],
                    -float(D),
                    kcol[:, s : s + 1],
                    op0=mybir.AluOpType.mult,
                    op1=mybir.AluOpType.add,
                )
                v2 = rpool.tile([128, 1], f32, tag="v2")
                nc.vector.tensor_add(v2[:], v1[:], sS[:])
                nc.vector.scalar_tensor_tensor(
                    acc[:],
                    v2[:],
                    wcol[:, s : s + 1],
                    acc[:],
                    op0=mybir.AluOpType.mult,
                    op1=mybir.AluOpType.add,
                )

            nc.sync.dma_start(out_d[:], acc[:])

    nc.compile()
    return nc


def kernel(points, sensitive_attribute, t):
    _install_ntff_hook()

    points = np.asarray(points, dtype=np.float32)
    sa = np.asarray(sensitive_attribute).astype(np.int64)
    n, d = points.shape
    assert d == D

    scale = 1.0 / math.sqrt(float(np.asarray(t)))
    order = np.argsort(sa, kind="stable")
    sa_sorted = sa[order]
    ps = (points[order] * np.float32(scale)).astype(np.float32)  # [n, 32] sorted
    ps_bf = ps.astype(ml_dtypes.bfloat16)

    # group size per sorted position (for the 1/P^2 weights)
    _, counts = np.unique(sa_sorted, return_counts=True)
    gsize = np.repeat(counts, counts).astype(np.float64)

    ntiles, per_core = _plan(sa_sorted)

    lnD = math.log(float(D))
    onesbd = np.zeros((128, 8 * 32), ml_dtypes.bfloat16)
    for i in range(8):
        for a in range(4):
            onesbd[32 * a : 32 * (a + 1), 32 * i + 4 * i + a] = 1.0

    in_maps = []
    for c in range(N_CORES):
        rep4 = np.zeros((128, ntiles * W), ml_dtypes.bfloat16)
        winj = np.zeros((32, ntiles * W), ml_dtypes.bfloat16)
        lhsa = np.zeros((32, ntiles * 128), ml_dtypes.bfloat16)
        scal2 = np.zeros((128, ntiles * 2 * PACKS), ml_dtypes.bfloat16)
        wk = np.zeros((128, 2 * ntiles), np.float32)
        wcol = wk[:, 0:ntiles]
        kcol = wk[:, ntiles : 2 * ntiles]
        for s, slot in enumerate(per_core[c]):
            if slot is None:
                # dummy slot: all-zero data; exp(0) rows sum to D, finite
                # log, zero weight.
                continue
            rows, wts, c0, L = slot
            R = len(rows)
            win = ps_bf[c0 : c0 + L].T  # [32, L]
            winj[:, s * W : s * W + L] = win
            rep4[:, s * W : s * W + L] = np.tile(win, (4, 1))
            ablk = np.zeros((32, 128), np.float32)
            ablk[:, :R] = ps[rows].T
            lhsa[:, s * 128 : (s + 1) * 128] = ablk.astype(ml_dtypes.bfloat16)
            # scal column k = slot rows 4k..4k+3 flattened (a-major,
            # d-minor), stored twice (pair duplication for DVE 2x)
            sc = ablk.T.reshape(PACKS, 128).T.astype(ml_dtypes.bfloat16)
            scal2[:, s * 2 * PACKS : (s + 1) * 2 * PACKS] = np.repeat(sc, 2, axis=1)
            P = gsize[rows]
            wcol[:R, s] = -wts / (n * P * P)
            kcol[:, s] = D * lnD * (W - L)

        in_maps.append(
            {
                "rep4": rep4,
                "winj": winj,
                "lhsa": lhsa,
                "scal2": scal2,
                "wk": wk,
                "onesbd": onesbd,
            }
        )

    nc = _build_program(ntiles)
    trace = bool(int(os.environ.get("KERNEL_TRACE", "0")))
    res = run_bass_kernel_spmd(nc, in_maps, list(range(N_CORES)), trace=trace)
    last_run_info["exec_time_ns"] = res.exec_time_ns
    last_run_info["mean_exec_time_ns"] = res.mean_exec_time_ns
    last_run_info["W"] = W
    last_run_info["ntiles"] = ntiles
    last_run_info["instructions"] = (
        res.instructions_and_trace[0] if res.instructions_and_trace else None
    )

    total = 0.0
    for c in range(N_CORES):
        total += float(res.results[c]["out"].astype(np.float64).sum())
    return np.float32(total)


# revision 18
# speedup vs baseline: 1.0073x; 1.0073x over previous
"""Trainium2 Bass kernel for the grouped contrastive loss.

Math: for anchors i and positives j restricted to the same
sensitive-attribute group g (size P),
    row(i,j) = S_ij - D * log E_ij
with S_ij = <p_i, p_j>/t and E_ij = sum_d exp(p_i[d] p_j[d] / t)
(the log-softmax max-shift cancels analytically), and
    loss = sum_i -1/(N P_i^2) * sum_{j in g(i)} row(i,j).

row(i,j) is symmetric, so the group's P x P matrix is covered by
chunking each group into <=128-column chunks: the diagonal chunk-square
is computed in full at weight 1 and cross chunk pairs only once (rows of
earlier chunks x cols of later chunk) at weight 2 -- B(B+1)/2 slots per
group instead of B^2. Slot = up to 128 anchor rows x one col chunk
(W=128), rows packed 128-at-a-time from all chunks <= the col chunk.

Per slot, on device (anchors on partitions as 32 packs of 4 anchors x
32 dims):
  - sum_j S_ij = <a_i, sum_j w_j> directly via one N=1 bf16 matmul
    against the host-precomputed window-sum vector (no [128,128] S tile
    at all), accumulated into a [128, ntiles] PSUM column tile.
  - prod via ONE DVE tensor_tensor per slot: scalars stored duplicated
    in pairs (scal2[p,2k]=scal2[p,2k+1]) so all three operands' APs end
    in a packed [1,2] bf16 dim -> DVE 2x mode; stride-0 outer dims do
    the pack/window broadcast. Then one batched ACT Exp ([128, 4096]
    bf16), and per-pack bf16 matmuls against shifted block-diagonal
    ones accumulating each anchor's 32 exp rows into its PSUM row
    (4 chains x 8 packs into one [128,128] PSUM tile via explicit
    tile_position). Slot 0 is sub-chunked 4x to shorten the startup
    ramp.
  - Ln on ACT; DVE row-reduction of log E into a [128, ntiles] column
    tile. A single epilogue applies the per-(row,slot) weights and
    reduces to the [128] per-core partial.
A manually pre-placed InstLoadActFuncSet of the combined exp+ln table
avoids the per-switch ACT table reloads. Dummy rows/cols are weighted
out (w=0); the exact -D*ln(D)*n_dummy-per-slot correction is added by
the host. The 8 cores run one SPMD program; the host sums the [128]
partials.
"""

import math
import os
import sys

sys.path.insert(0, "/opt/trn_rl_repo")

import numpy as np
import ml_dtypes

import concourse.bacc as bacc
import concourse.bass as bass
import concourse.tile as tile
from concourse import mybir
from concourse.bass_utils import run_bass_kernel_spmd

N_CORES = 8
D = 32
W = 128  # window (col chunk) width
PACKS = 32  # packs of 4 anchors per 128-anchor slot

last_run_info = {}


def _install_ntff_hook():
    # bass_utils' trace path under axon imports antenv.axon_hooks, which is
    # absent in this image; provide the ctypes-based hook it expects.
    import contextlib
    import ctypes
    import types

    if "antenv.axon_hooks" in sys.modules:
        return

    def _make_hook():
        try:
            lib = ctypes.CDLL("/opt/axon/libaxon_pjrt.so")
        except OSError:
            return None
        if not hasattr(lib, "axon_start_nrt_profile"):
            return None
        lib.axon_start_nrt_profile.argtypes = [
            ctypes.POINTER(ctypes.c_int64),
            ctypes.c_size_t,
        ]
        lib.axon_start_nrt_profile.restype = ctypes.c_int64
        lib.axon_stop_nrt_profile.argtypes = [ctypes.c_char_p]
        lib.axon_stop_nrt_profile.restype = ctypes.c_int64

        @contextlib.contextmanager
        def _hook_cm(output_dir, device_ids):
            import jax

            jax.devices()
            if device_ids:
                ids = (ctypes.c_int64 * len(device_ids))(*device_ids)
                rc = lib.axon_start_nrt_profile(ids, len(device_ids))
            else:
                rc = lib.axon_start_nrt_profile(None, 0)
            if rc != 0:
                raise RuntimeError(f"axon_start_nrt_profile rc={rc}")
            try:
                yield
            finally:
                n = lib.axon_stop_nrt_profile(str(output_dir).encode())
                if n < 0:
                    raise RuntimeError(f"axon_stop_nrt_profile rc={n}")

        return _hook_cm

    hook = _make_hook()
    mod = types.ModuleType("antenv.axon_hooks")
    mod.get_axon_ntff_profile_hook = lambda: hook
    mod.set_axon_ntff_profile_hook = lambda h: None
    sys.modules["antenv.axon_hooks"] = mod


def _plan(sa_sorted):
    """Slot plan from the sorted attribute vector.

    Each slot is (rows, weights, c0, L):
      rows: array of <=128 sorted-anchor positions (the slot's anchors)
      weights: per-row pair multiplicity (1 diag chunk, 2 earlier chunk)
      [c0, c0+L): the slot's col window (sorted positions, one chunk)
    or None for a dummy slot. Returns (ntiles, per_core).
    """
    n = len(sa_sorted)
    bounds = [0]
    for i in range(1, n):
        if sa_sorted[i] != sa_sorted[i - 1]:
            bounds.append(i)
    bounds.append(n)

    slots = []
    for gi in range(len(bounds) - 1):
        g0, g1 = bounds[gi], bounds[gi + 1]
        P = g1 - g0
        B = (P + W - 1) // W
        for w in range(B):
            c0 = g0 + W * w
            L = min(W, g1 - c0)
            r_hi = min(g0 + W * (w + 1), g1)  # rows of chunks 0..w
            rows_all = np.arange(g0, r_hi)
            wts_all = np.where(rows_all < c0, 2.0, 1.0)
            for r0 in range(0, len(rows_all), 128):
                slots.append(
                    (rows_all[r0 : r0 + 128], wts_all[r0 : r0 + 128], c0, L)
                )

    ntiles = (len(slots) + N_CORES - 1) // N_CORES
    per_core = [[] for _ in range(N_CORES)]
    for i, s in enumerate(slots):
        per_core[i % N_CORES].append(s)
    for c in range(N_CORES):
        while len(per_core[c]) < ntiles:
            per_core[c].append(None)
    return ntiles, per_core


def _exp_ln_table_id(nc):
    try:
        from concourse.hw_specs import get_activation_tables

        tabs = get_activation_tables(nc.m.arch)
        Exp = mybir.ActivationFunctionType.Exp
        Ln = mybir.ActivationFunctionType.Ln
        for idx, funcs in enumerate(tabs.values()):
            if Exp in funcs and Ln in funcs:
                return idx
    except Exception:
        pass
    return 6  # natural_log_exp_and_others in this neuronxcc's act_info.json


def _build_program(ntiles):
    # Bacc (not raw Bass): its compile() runs generate_event_semaphores,
    # which splits multi-semaphore waits to satisfy the TRN2 one-wait-per-
    # instruction constraint this walrus build enforces.
    nc = bacc.Bacc(
        "TRN2", target_bir_lowering=False, debug=False, num_devices=N_CORES
    )
    f32 = mybir.dt.float32
    bf16 = mybir.dt.bfloat16

    rep4_d = nc.dram_tensor("rep4", [128, ntiles * W], bf16, kind="ExternalInput").ap()
    wsum_d = nc.dram_tensor("wsums", [32, ntiles], bf16, kind="ExternalInput").ap()
    lhsa_d = nc.dram_tensor("lhsa", [32, ntiles * 128], bf16, kind="ExternalInput").ap()
    scal_d = nc.dram_tensor(
        "scal2", [128, ntiles * 2 * PACKS], bf16, kind="ExternalInput"
    ).ap()
    # AB packs A=wcol (cols 0:ntiles) and B=-D*wcol (cols ntiles:2*ntiles)
    ab_d = nc.dram_tensor("AB", [128, 2 * ntiles], f32, kind="ExternalInput").ap()
    ones_d = nc.dram_tensor("onesbd", [128, 8 * 32], bf16, kind="ExternalInput").ap()
    out_d = nc.dram_tensor("out", [128, 1], f32, kind="ExternalOutput").ap()

    Exp = mybir.ActivationFunctionType.Exp
    Ln = mybir.ActivationFunctionType.Ln

    with tile.TileContext(nc) as tc:
        with (
            tc.tile_pool(name="const", bufs=1) as cpool,
            tc.tile_pool(name="work", bufs=3) as wpool,
            tc.tile_pool(name="red", bufs=1) as rpool,
            tc.tile_pool(name="psE", bufs=3, space="PSUM") as psE,
            tc.tile_pool(name="psL", bufs=3, space="PSUM") as psL,
            tc.tile_pool(name="psS", bufs=1, space="PSUM") as psS,
        ):
            # preload the combined exp+ln table so Exp/Ln interleaving
            # never reloads activation tables (saves ~1.3us per switch)
            nc.scalar.add_instruction(
                mybir.InstLoadActFuncSet(
                    name=nc.get_next_instruction_name(),
                    ins=[],
                    outs=[],
                    act_func_set_id=_exp_ln_table_id(nc),
                )
            )

            # slot-0 slices land first (parallel issue on gpsimd/sync) so
            # compute starts before the bulk transfers
            rep4 = cpool.tile([128, ntiles * W], bf16, tag="rep4")
            nc.gpsimd.dma_start(rep4[:, 0:W], rep4_d[:, 0:W])
            scal = cpool.tile([128, ntiles * 2 * PACKS], bf16, tag="scal2")
            nc.sync.dma_start(scal[:, 0 : 2 * PACKS], scal_d[:, 0 : 2 * PACKS])
            nc.gpsimd.dma_start(rep4[:, W:], rep4_d[:, W:])
            nc.sync.dma_start(scal[:, 2 * PACKS :], scal_d[:, 2 * PACKS :])
            lhsa = cpool.tile([32, ntiles * 128], bf16, tag="lhsa")
            nc.scalar.dma_start(lhsa[:], lhsa_d[:])
            wsums = cpool.tile([32, ntiles], bf16, tag="wsums")
            nc.scalar.dma_start(wsums[:], wsum_d[:])
            onesbd = cpool.tile([128, 8 * 32], bf16, tag="onesbd")
            nc.gpsimd.dma_start(onesbd[:], ones_d[:])
            ab = cpool.tile([128, 2 * ntiles], f32, tag="AB")
            nc.sync.dma_start(ab[:], ab_d[:])

            SL = cpool.tile([128, ntiles], f32, tag="SL")
            SS = psS.tile([128, ntiles], f32, tag="SS")

            def mult_exp(s, k0, k1, prod, expt):
                # prod[:, k*W:(k+1)*W] = rep4_s * scal_s[:,k] for k0<=k<k1,
                # in one 2x-mode DVE op via pair-duplicated scalars
                nk = k1 - k0
                in0 = (
                    rep4[:, s * W : (s + 1) * W]
                    .rearrange("p (j2 two) -> p j2 two", two=2)
                    .unsqueeze(1)
                    .broadcast_to([128, nk, W // 2, 2])
                )
                in1 = (
                    scal[:, s * 2 * PACKS + 2 * k0 : s * 2 * PACKS + 2 * k1]
                    .rearrange("p (k two) -> p k two", two=2)
                    .unsqueeze(2)
                    .broadcast_to([128, nk, W // 2, 2])
                )
                outp = prod[:, k0 * W : k1 * W].rearrange(
                    "p (k j2 two) -> p k j2 two", k=nk, two=2
                )
                nc.vector.tensor_tensor(outp, in0, in1, op=mybir.AluOpType.mult)
                nc.scalar.activation(
                    expt[:, k0 * W : k1 * W], prod[:, k0 * W : k1 * W], Exp
                )

            def e_chain(h, E_ps, expt):
                for i in range(8):
                    k = 8 * h + i
                    nc.tensor.matmul(
                        E_ps[32 * h : 32 * h + 32, :],
                        lhsT=onesbd[:, 32 * i : 32 * (i + 1)],
                        rhs=expt[:, k * W : (k + 1) * W],
                        start=(i == 0),
                        stop=(i == 7),
                        tile_position=(0, 32 * h),
                    )

            for s in range(ntiles):
                nc.tensor.matmul(
                    SS[:, s : s + 1],
                    lhsT=lhsa[:, s * 128 : (s + 1) * 128],
                    rhs=wsums[:, s : s + 1],
                    start=True,
                    stop=True,
                )
                prod = wpool.tile([128, PACKS * W], bf16, tag="prod")
                expt = wpool.tile([128, PACKS * W], bf16, tag="expt")
                E_ps = psE.tile([128, W], f32, tag="E")
                if s == 0:
                    for h in range(4):
                        mult_exp(s, 8 * h, 8 * (h + 1), prod, expt)
                        e_chain(h, E_ps, expt)
                else:
                    mult_exp(s, 0, PACKS, prod, expt)
                    for h in range(4):
                        e_chain(h, E_ps, expt)
                logE = psL.tile([128, W], f32, tag="logE")
                nc.scalar.activation(logE[:], E_ps[:], Ln)
                nc.vector.tensor_reduce(
                    SL[:, s : s + 1],
                    logE[:],
                    axis=mybir.AxisListType.X,
                    op=mybir.AluOpType.add,
                )

            # epilogue: acc = sum_s A*ssb + B*SL
            ssb = rpool.tile([128, ntiles], f32, tag="ssb")
            nc.vector.tensor_copy(ssb[:], SS[:])
            u1 = rpool.tile([128, ntiles], f32, tag="u1")
            nc.vector.tensor_tensor(
                u1[:], ab[:, 0:ntiles], ssb[:], op=mybir.AluOpType.mult
            )
            u2 = rpool.tile([128, ntiles], f32, tag="u2")
            nc.vector.tensor_tensor(
                u2[:], ab[:, ntiles : 2 * ntiles], SL[:], op=mybir.AluOpType.mult
            )
            u3 = rpool.tile([128, ntiles], f32, tag="u3")
            nc.vector.tensor_add(u3[:], u1[:], u2[:])
            acc = rpool.tile([128, 1], f32, tag="acc")
            nc.vector.tensor_reduce(
                acc[:], u3[:], axis=mybir.AxisListType.X, op=mybir.AluOpType.add
            )
            nc.sync.dma_start(out_d[:], acc[:])

    nc.compile()
    return nc


def kernel(points, sensitive_attribute, t):
    _install_ntff_hook()

    points = np.asarray(points, dtype=np.float32)
    sa = np.asarray(sensitive_attribute).astype(np.int64)
    n, d = points.shape
    assert d == D

    scale = 1.0 / math.sqrt(float(np.asarray(t)))
    order = np.argsort(sa, kind="stable")
    sa_sorted = sa[order]
    ps = (points[order] * np.float32(scale)).astype(np.float32)  # [n, 32] sorted
    ps_bf = ps.astype(ml_dtypes.bfloat16)

    # group size per sorted position (for the 1/P^2 weights)
    _, counts = np.unique(sa_sorted, return_counts=True)
    gsize = np.repeat(counts, counts).astype(np.float64)

    ntiles, per_core = _plan(sa_sorted)

    lnD = math.log(float(D))
    onesbd = np.zeros((128, 8 * 32), ml_dtypes.bfloat16)
    for i in range(8):
        for a in range(4):
            onesbd[32 * a : 32 * (a + 1), 32 * i + 4 * i + a] = 1.0

    in_maps = []
    host_const = 0.0  # sum of per-row dummy-col corrections (exact)
    for c in range(N_CORES):
        rep4 = np.zeros((128, ntiles * W), ml_dtypes.bfloat16)
        wsums = np.zeros((32, ntiles), ml_dtypes.bfloat16)
        lhsa = np.zeros((32, ntiles * 128), ml_dtypes.bfloat16)
        scal2 = np.zeros((128, ntiles * 2 * PACKS), ml_dtypes.bfloat16)
        AB = np.zeros((128, 2 * ntiles), np.float32)
        for s, slot in enumerate(per_core[c]):
            if slot is None:
                # dummy slot: all-zero data; exp(0) rows sum to D, finite
                # log, zero weight.
                continue
            rows, wts, c0, L = slot
            R = len(rows)
            win = ps_bf[c0 : c0 + L].T  # [32, L]
            rep4[:, s * W : s * W + L] = np.tile(win, (4, 1))
            wsums[:, s] = win.astype(np.float32).sum(axis=1).astype(
                ml_dtypes.bfloat16
            )
            ablk = np.zeros((32, 128), np.float32)
            ablk[:, :R] = ps[rows].T
            lhsa[:, s * 128 : (s + 1) * 128] = ablk.astype(ml_dtypes.bfloat16)
            # scal column k = slot rows 4k..4k+3 flattened (a-major,
            # d-minor), stored twice (pair duplication for DVE 2x)
            sc = ablk.T.reshape(PACKS, 128).T.astype(ml_dtypes.bfloat16)
            scal2[:, s * 2 * PACKS : (s + 1) * 2 * PACKS] = np.repeat(sc, 2, axis=1)
            P = gsize[rows]
            wcol_r = -wts / (n * P * P)  # [R]
            AB[:R, s] = wcol_r
            AB[:R, ntiles + s] = -float(D) * wcol_r
            # dummy-col correction, applied host-side:
            # each real row picks up (W-L) * (-D*lnD) at weight wcol
            host_const += float(np.sum(wcol_r) * (D * lnD) * (W - L))

        in_maps.append(
            {
                "rep4": rep4,
                "wsums": wsums,
                "lhsa": lhsa,
                "scal2": scal2,
                "AB": AB,
                "onesbd": onesbd,
            }
        )

    nc = _build_program(ntiles)
    trace = bool(int(os.environ.get("KERNEL_TRACE", "0")))
    res = run_bass_kernel_spmd(nc, in_maps, list(range(N_CORES)), trace=trace)
    last_run_info["exec_time_ns"] = res.exec_time_ns
    last_run_info["mean_exec_time_ns"] = res.mean_exec_time_ns
    last_run_info["W"] = W
    last_run_info["ntiles"] = ntiles
    last_run_info["instructions"] = (
        res.instructions_and_trace[0] if res.instructions_and_trace else None
    )

    total = float(host_const)
    for c in range(N_CORES):
        total += float(res.results[c]["out"].astype(np.float64).sum())
    return np.float32(total)


# revision 25
# speedup vs baseline: 1.1102x; 1.1021x over previous
"""Trainium2 Bass kernel for the grouped contrastive loss.

Math: for anchors i and positives j restricted to the same
sensitive-attribute group g (size P),
    row(i,j) = S_ij - D * log E_ij
with S_ij = <p_i, p_j>/t and E_ij = sum_d exp(p_i[d] p_j[d] / t)
(the log-softmax max-shift cancels analytically), and
    loss = sum_i -1/(N P_i^2) * sum_{j in g(i)} row(i,j).

row(i,j) is symmetric, so the group's P x P matrix is covered by
chunking each group into <=128-column chunks: the diagonal chunk-square
is computed in full at weight 1 and cross chunk pairs only once (rows of
earlier chunks x cols of later chunk) at weight 2 -- B(B+1)/2 slots per
group instead of B^2. Slot = up to 128 anchor rows x one col chunk
(W=128), rows packed 128-at-a-time from all chunks <= the col chunk.

Per slot, on device (anchors on partitions as 32 packs of 4 anchors x
32 dims):
  - sum_j S_ij = <a_i, sum_j w_j> directly via one N=1 bf16 matmul
    against the host-precomputed window-sum vector (no [128,128] S tile
    at all), accumulated into a [128, ntiles] PSUM column tile.
  - prod via ONE DVE tensor_tensor per slot: scalars stored duplicated
    in pairs (scal2[p,2k]=scal2[p,2k+1]) so all three operands' APs end
    in a packed [1,2] bf16 dim -> DVE 2x mode; stride-0 outer dims do
    the pack/window broadcast. Then one batched ACT Exp ([128, 4096]
    bf16), and per-pack bf16 matmuls against shifted block-diagonal
    ones accumulating each anchor's 32 exp rows into its PSUM row
    (4 chains x 8 packs into one [128,128] PSUM tile via explicit
    tile_position). Slot 0 is sub-chunked 4x to shorten the startup
    ramp.
  - Ln on ACT; DVE row-reduction of log E into a [128, ntiles] column
    tile. A single epilogue applies the per-(row,slot) weights and
    reduces to the [128] per-core partial.
A manually pre-placed InstLoadActFuncSet of the combined exp+ln table
avoids the per-switch ACT table reloads. Dummy rows/cols are weighted
out (w=0); the exact -D*ln(D)*n_dummy-per-slot correction is added by
the host. The 8 cores run one SPMD program; the host sums the [128]
partials.
"""

import math
import os
import sys

sys.path.insert(0, "/opt/trn_rl_repo")

import numpy as np
import ml_dtypes

import concourse.bacc as bacc
import concourse.bass as bass
import concourse.tile as tile
from concourse import mybir
from concourse.bass_utils import run_bass_kernel_spmd

N_CORES = 8
D = 32
W = 128  # window (col chunk) width
PACKS = 32  # packs of 4 anchors per 128-anchor slot

last_run_info = {}


def _install_ntff_hook():
    # bass_utils' trace path under axon imports antenv.axon_hooks, which is
    # absent in this image; provide the ctypes-based hook it expects.
    import contextlib
    import ctypes
    import types

    if "antenv.axon_hooks" in sys.modules:
        return

    def _make_hook():
        try:
            lib = ctypes.CDLL("/opt/axon/libaxon_pjrt.so")
        except OSError:
            return None
        if not hasattr(lib, "axon_start_nrt_profile"):
            return None
        lib.axon_start_nrt_profile.argtypes = [
            ctypes.POINTER(ctypes.c_int64),
            ctypes.c_size_t,
        ]
        lib.axon_start_nrt_profile.restype = ctypes.c_int64
        lib.axon_stop_nrt_profile.argtypes = [ctypes.c_char_p]
        lib.axon_stop_nrt_profile.restype = ctypes.c_int64

        @contextlib.contextmanager
        def _hook_cm(output_dir, device_ids):
            import jax

            jax.devices()
            if device_ids:
                ids = (ctypes.c_int64 * len(device_ids))(*device_ids)
                rc = lib.axon_start_nrt_profile(ids, len(device_ids))
            else:
                rc = lib.axon_start_nrt_profile(None, 0)
            if rc != 0:
                raise RuntimeError(f"axon_start_nrt_profile rc={rc}")
            try:
                yield
            finally:
                n = lib.axon_stop_nrt_profile(str(output_dir).encode())
                if n < 0:
                    raise RuntimeError(f"axon_stop_nrt_profile rc={n}")

        return _hook_cm

    hook = _make_hook()
    mod = types.ModuleType("antenv.axon_hooks")
    mod.get_axon_ntff_profile_hook = lambda: hook
    mod.set_axon_ntff_profile_hook = lambda h: None
    sys.modules["antenv.axon_hooks"] = mod


def _plan(sa_sorted):
    """Slot plan from the sorted attribute vector.

    Each slot is (rows, weights, c0, L):
      rows: array of <=128 sorted-anchor positions (the slot's anchors)
      weights: per-row pair multiplicity (1 diag chunk, 2 earlier chunk)
      [c0, c0+L): the slot's col window (sorted positions, one chunk)
    or None for a dummy slot. Returns (ntiles, per_core).
    """
    n = len(sa_sorted)
    bounds = [0]
    for i in range(1, n):
        if sa_sorted[i] != sa_sorted[i - 1]:
            bounds.append(i)
    bounds.append(n)

    slots = []
    for gi in range(len(bounds) - 1):
        g0, g1 = bounds[gi], bounds[gi + 1]
        P = g1 - g0
        B = (P + W - 1) // W
        for w in range(B):
            c0 = g0 + W * w
            L = min(W, g1 - c0)
            r_hi = min(g0 + W * (w + 1), g1)  # rows of chunks 0..w
            rows_all = np.arange(g0, r_hi)
            wts_all = np.where(rows_all < c0, 2.0, 1.0)
            for r0 in range(0, len(rows_all), 128):
                slots.append(
                    (rows_all[r0 : r0 + 128], wts_all[r0 : r0 + 128], c0, L)
                )

    # floor(S/8) full slots per core; the leftover slots are split
    # column-wise into per-core mini-pieces of width Lc (the cores run
    # the same program; the piece data differs per core).
    S = len(slots)
    ntiles = S // N_CORES
    leftover = S - ntiles * N_CORES
    full = slots[: ntiles * N_CORES]
    rest = slots[ntiles * N_CORES :]
    per_core = [[] for _ in range(N_CORES)]
    for i, s in enumerate(full):
        per_core[i % N_CORES].append(s)

    minis = [None] * N_CORES
    Lc = 0
    if leftover:
        m = next(v for v in (1, 2, 4, 8) if v >= leftover)
        cps = N_CORES // m  # cores per leftover slot
        Lc = W // cps
        for c in range(N_CORES):
            j = c // cps
            q = c % cps
            if j < leftover:
                rows, wts, c0, L = rest[j]
                off = q * Lc
                L2 = max(0, min(Lc, L - off))
                minis[c] = (rows, wts, c0 + off, L2)
    return ntiles, per_core, minis, Lc


def _exp_ln_table_id(nc):
    try:
        from concourse.hw_specs import get_activation_tables

        tabs = get_activation_tables(nc.m.arch)
        Exp = mybir.ActivationFunctionType.Exp
        Ln = mybir.ActivationFunctionType.Ln
        for idx, funcs in enumerate(tabs.values()):
            if Exp in funcs and Ln in funcs:
                return idx
    except Exception:
        pass
    return 6  # natural_log_exp_and_others in this neuronxcc's act_info.json


def _build_program(ntiles, Lc):
    # Bacc (not raw Bass): its compile() runs generate_event_semaphores,
    # which splits multi-semaphore waits to satisfy the TRN2 one-wait-per-
    # instruction constraint this walrus build enforces.
    nc = bacc.Bacc(
        "TRN2", target_bir_lowering=False, debug=False, num_devices=N_CORES
    )
    f32 = mybir.dt.float32
    bf16 = mybir.dt.bfloat16

    nmini = 1 if Lc else 0
    ncols = ntiles + nmini

    rep4_d = nc.dram_tensor("rep4", [128, ntiles * W], bf16, kind="ExternalInput").ap()
    wsum_d = nc.dram_tensor("wsums", [32, ntiles], bf16, kind="ExternalInput").ap()
    lhsa_d = nc.dram_tensor("lhsa", [32, ntiles * 128], bf16, kind="ExternalInput").ap()
    scal_d = nc.dram_tensor(
        "scal2", [128, ntiles * 2 * PACKS], bf16, kind="ExternalInput"
    ).ap()
    # AB packs A=wcol (cols 0:ncols) and B=-D*wcol (cols ncols:2*ncols)
    ab_d = nc.dram_tensor("AB", [128, 2 * ncols], f32, kind="ExternalInput").ap()
    ones_d = nc.dram_tensor("onesbd", [128, 8 * 32], bf16, kind="ExternalInput").ap()
    if Lc:
        # mini piece: rep4m (cols 0:Lc) + scal2m (cols Lc:Lc+64)
        minib_d = nc.dram_tensor(
            "minibig", [128, Lc + 2 * PACKS], bf16, kind="ExternalInput"
        ).ap()
        # lhsam (cols 0:128) + wsumm (col 128)
        minia_d = nc.dram_tensor("minia", [32, 129], bf16, kind="ExternalInput").ap()
    out_d = nc.dram_tensor("out", [128, 1], f32, kind="ExternalOutput").ap()

    Exp = mybir.ActivationFunctionType.Exp
    Ln = mybir.ActivationFunctionType.Ln

    with tile.TileContext(nc) as tc:
        with (
            tc.tile_pool(name="const", bufs=1) as cpool,
            tc.tile_pool(name="work", bufs=3) as wpool,
            tc.tile_pool(name="red", bufs=1) as rpool,
            tc.tile_pool(name="psE", bufs=3, space="PSUM") as psE,
            tc.tile_pool(name="psL", bufs=3, space="PSUM") as psL,
            tc.tile_pool(name="psS", bufs=1, space="PSUM") as psS,
        ):
            # preload the combined exp+ln table so Exp/Ln interleaving
            # never reloads activation tables (saves ~1.3us per switch)
            nc.scalar.add_instruction(
                mybir.InstLoadActFuncSet(
                    name=nc.get_next_instruction_name(),
                    ins=[],
                    outs=[],
                    act_func_set_id=_exp_ln_table_id(nc),
                )
            )

            # slot-0 slices land first (parallel issue on gpsimd/sync) so
            # compute starts before the bulk transfers
            rep4 = cpool.tile([128, ntiles * W], bf16, tag="rep4")
            nc.gpsimd.dma_start(rep4[:, 0:W], rep4_d[:, 0:W])
            scal = cpool.tile([128, ntiles * 2 * PACKS], bf16, tag="scal2")
            nc.sync.dma_start(scal[:, 0 : 2 * PACKS], scal_d[:, 0 : 2 * PACKS])
            nc.gpsimd.dma_start(rep4[:, W:], rep4_d[:, W:])
            nc.sync.dma_start(scal[:, 2 * PACKS :], scal_d[:, 2 * PACKS :])
            lhsa = cpool.tile([32, ntiles * 128], bf16, tag="lhsa")
            nc.scalar.dma_start(lhsa[:], lhsa_d[:])
            wsums = cpool.tile([32, ntiles], bf16, tag="wsums")
            nc.scalar.dma_start(wsums[:], wsum_d[:])
            onesbd = cpool.tile([128, 8 * 32], bf16, tag="onesbd")
            nc.gpsimd.dma_start(onesbd[:], ones_d[:])
            ab = cpool.tile([128, 2 * ncols], f32, tag="AB")
            nc.sync.dma_start(ab[:], ab_d[:])
            if Lc:
                minib = cpool.tile([128, Lc + 2 * PACKS], bf16, tag="minibig")
                nc.gpsimd.dma_start(minib[:], minib_d[:])
                minia = cpool.tile([32, 129], bf16, tag="minia")
                nc.scalar.dma_start(minia[:], minia_d[:])

            SL = cpool.tile([128, ncols], f32, tag="SL")
            SS = psS.tile([128, ncols], f32, tag="SS")

            def mult_exp(rep_ap, scal_ap, k0, k1, prod, expt, width):
                # prod[:, k*width:(k+1)*width] = rep * scal[:,k] for
                # k0<=k<k1, in one 2x-mode DVE op via pair-dup scalars
                nk = k1 - k0
                in0 = (
                    rep_ap.rearrange("p (j2 two) -> p j2 two", two=2)
                    .unsqueeze(1)
                    .broadcast_to([128, nk, width // 2, 2])
                )
                in1 = (
                    scal_ap[:, 2 * k0 : 2 * k1]
                    .rearrange("p (k two) -> p k two", two=2)
                    .unsqueeze(2)
                    .broadcast_to([128, nk, width // 2, 2])
                )
                outp = prod[:, k0 * width : k1 * width].rearrange(
                    "p (k j2 two) -> p k j2 two", k=nk, two=2
                )
                nc.vector.tensor_tensor(outp, in0, in1, op=mybir.AluOpType.mult)
                nc.scalar.activation(
                    expt[:, k0 * width : k1 * width],
                    prod[:, k0 * width : k1 * width],
                    Exp,
                )

            def e_chain(h, E_ps, expt, width):
                for i in range(8):
                    k = 8 * h + i
                    nc.tensor.matmul(
                        E_ps[32 * h : 32 * h + 32, :],
                        lhsT=onesbd[:, 32 * i : 32 * (i + 1)],
                        rhs=expt[:, k * width : (k + 1) * width],
                        start=(i == 0),
                        stop=(i == 7),
                        tile_position=(0, 32 * h),
                    )

            def slot_body(col, rep_ap, scal_ap, lhsa_ap, wsum_ap, width, nsub):
                nc.tensor.matmul(
                    SS[:, col : col + 1],
                    lhsT=lhsa_ap,
                    rhs=wsum_ap,
                    start=True,
                    stop=True,
                )
                prod = wpool.tile([128, PACKS * width], bf16, tag=f"prod{width}")
                expt = wpool.tile([128, PACKS * width], bf16, tag=f"expt{width}")
                E_full = psE.tile([128, W], f32, tag="E")
                E_ps = E_full[:, 0:width]
                if nsub == 4:
                    for h in range(4):
                        mult_exp(rep_ap, scal_ap, 8 * h, 8 * (h + 1), prod, expt, width)
                        e_chain(h, E_ps, expt, width)
                else:
                    mult_exp(rep_ap, scal_ap, 0, PACKS, prod, expt, width)
                    for h in range(4):
                        e_chain(h, E_ps, expt, width)
                logE = psL.tile([128, W], f32, tag="logE")
                nc.scalar.activation(logE[:, 0:width], E_ps, Ln)
                nc.vector.tensor_reduce(
                    SL[:, col : col + 1],
                    logE[:, 0:width],
                    axis=mybir.AxisListType.X,
                    op=mybir.AluOpType.add,
                )

            for s in range(ntiles):
                slot_body(
                    s,
                    rep4[:, s * W : (s + 1) * W],
                    scal[:, s * 2 * PACKS : (s + 1) * 2 * PACKS],
                    lhsa[:, s * 128 : (s + 1) * 128],
                    wsums[:, s : s + 1],
                    W,
                    4 if s == 0 else 1,
                )
            if Lc:
                slot_body(
                    ntiles,
                    minib[:, 0:Lc],
                    minib[:, Lc : Lc + 2 * PACKS],
                    minia[:, 0:128],
                    minia[:, 128:129],
                    Lc,
                    1,
                )

            # epilogue: acc = sum_s A*ssb + B*SL
            ssb = rpool.tile([128, ncols], f32, tag="ssb")
            nc.vector.tensor_copy(ssb[:], SS[:])
            u1 = rpool.tile([128, ncols], f32, tag="u1")
            nc.vector.tensor_tensor(
                u1[:], ab[:, 0:ncols], ssb[:], op=mybir.AluOpType.mult
            )
            u2 = rpool.tile([128, ncols], f32, tag="u2")
            nc.vector.tensor_tensor(
                u2[:], ab[:, ncols : 2 * ncols], SL[:], op=mybir.AluOpType.mult
            )
            u3 = rpool.tile([128, ncols], f32, tag="u3")
            nc.vector.tensor_add(u3[:], u1[:], u2[:])
            acc = rpool.tile([128, 1], f32, tag="acc")
            nc.vector.tensor_reduce(
                acc[:], u3[:], axis=mybir.AxisListType.X, op=mybir.AluOpType.add
            )
            nc.sync.dma_start(out_d[:], acc[:])

    nc.compile()
    return nc


def kernel(points, sensitive_attribute, t):
    _install_ntff_hook()

    points = np.asarray(points, dtype=np.float32)
    sa = np.asarray(sensitive_attribute).astype(np.int64)
    n, d = points.shape
    assert d == D

    scale = 1.0 / math.sqrt(float(np.asarray(t)))
    order = np.argsort(sa, kind="stable")
    sa_sorted = sa[order]
    ps = (points[order] * np.float32(scale)).astype(np.float32)  # [n, 32] sorted
    ps_bf = ps.astype(ml_dtypes.bfloat16)

    # group size per sorted position (for the 1/P^2 weights)
    _, counts = np.unique(sa_sorted, return_counts=True)
    gsize = np.repeat(counts, counts).astype(np.float64)

    ntiles, per_core, minis, Lc = _plan(sa_sorted)
    nmini = 1 if Lc else 0
    ncols = ntiles + nmini

    lnD = math.log(float(D))
    onesbd = np.zeros((128, 8 * 32), ml_dtypes.bfloat16)
    for i in range(8):
        for a in range(4):
            onesbd[32 * a : 32 * (a + 1), 32 * i + 4 * i + a] = 1.0

    in_maps = []
    host_const = 0.0  # sum of per-row dummy-col corrections (exact)

    def pack_slot(slot, width):
        """-> (rep [128,width], sc2 [128,64], ablk16 [32,128], wsum [32],
        wcol_r [R], correction)"""
        rows, wts, c0, L = slot
        R = len(rows)
        rep = np.zeros((128, width), ml_dtypes.bfloat16)
        win = ps_bf[c0 : c0 + L].T  # [32, L]
        rep[:, :L] = np.tile(win, (4, 1))
        wsum = win.astype(np.float32).sum(axis=1).astype(ml_dtypes.bfloat16)
        ablk = np.zeros((32, 128), np.float32)
        ablk[:, :R] = ps[rows].T
        # scal column k = slot rows 4k..4k+3 flattened (a-major, d-minor),
        # stored twice (pair duplication for DVE 2x)
        sc = ablk.T.reshape(PACKS, 128).T.astype(ml_dtypes.bfloat16)
        P = gsize[rows]
        wcol_r = -wts / (n * P * P)
        corr = float(np.sum(wcol_r) * (D * lnD) * (width - L))
        return rep, np.repeat(sc, 2, axis=1), ablk.astype(
            ml_dtypes.bfloat16
        ), wsum, wcol_r, corr

    for c in range(N_CORES):
        rep4 = np.zeros((128, ntiles * W), ml_dtypes.bfloat16)
        wsums = np.zeros((32, ntiles), ml_dtypes.bfloat16)
        lhsa = np.zeros((32, ntiles * 128), ml_dtypes.bfloat16)
        scal2 = np.zeros((128, ntiles * 2 * PACKS), ml_dtypes.bfloat16)
        AB = np.zeros((128, 2 * ncols), np.float32)
        for s, slot in enumerate(per_core[c]):
            rep, sc2, ablk16, wsum, wcol_r, corr = pack_slot(slot, W)
            R = len(wcol_r)
            rep4[:, s * W : (s + 1) * W] = rep
            wsums[:, s] = wsum
            lhsa[:, s * 128 : (s + 1) * 128] = ablk16
            scal2[:, s * 2 * PACKS : (s + 1) * 2 * PACKS] = sc2
            AB[:R, s] = wcol_r
            AB[:R, ncols + s] = -float(D) * wcol_r
            host_const += corr

        im = {
            "rep4": rep4,
            "wsums": wsums,
            "lhsa": lhsa,
            "scal2": scal2,
            "AB": AB,
            "onesbd": onesbd,
        }
        if Lc:
            minibig = np.zeros((128, Lc + 2 * PACKS), ml_dtypes.bfloat16)
            minia = np.zeros((32, 129), ml_dtypes.bfloat16)
            if minis[c] is not None:
                rep, sc2, ablk16, wsum, wcol_r, corr = pack_slot(minis[c], Lc)
                R = len(wcol_r)
                minibig[:, 0:Lc] = rep
                minibig[:, Lc : Lc + 2 * PACKS] = sc2
                minia[:, 0:128] = ablk16
                minia[:, 128] = wsum
                AB[:R, ntiles] = wcol_r
                AB[:R, ncols + ntiles] = -float(D) * wcol_r
                host_const += corr
            im["minibig"] = minibig
            im["minia"] = minia
        in_maps.append(im)

    nc = _build_program(ntiles, Lc)
    trace = bool(int(os.environ.get("KERNEL_TRACE", "0")))
    res = run_bass_kernel_spmd(nc, in_maps, list(range(N_CORES)), trace=trace)
    last_run_info["exec_time_ns"] = res.exec_time_ns
    last_run_info["mean_exec_time_ns"] = res.mean_exec_time_ns
    last_run_info["W"] = W
    last_run_info["ntiles"] = ntiles
    last_run_info["instructions"] = (
        res.instructions_and_trace[0] if res.instructions_and_trace else None
    )

    total = float(host_const)
    for c in range(N_CORES):
        total += float(res.results[c]["out"].astype(np.float64).sum())
    return np.float32(total)
